# revision 1
# baseline (speedup 1.0000x reference)
"""Trainium2 Bass kernel for nn_Conv_39273180955618.

The reference op reduces to a depthwise correlation: every image (batch x
channel plane) of X is correlated with the same 3x3 kernel
Keff = K.sum((0,1)), plus a scalar bias b * prod(K.shape).

Strategy (8 NeuronCores, data-parallel over batch):
  - core k gets batches [2k, 2k+2) = 128 images of 224x224.
  - Per core, images are processed in blocks of IB images x 112-row chunks.
    Rows live on SBUF partitions, W stays contiguous on the free axis.
  - The H-convolution is a TensorE matmul contraction over rows with small
    banded matrices B[chunk, dw] (shape [113, 112]): for each of the 3 W
    shifts dw, Z[:, wout] += B^T @ X[rows, win], accumulated in PSUM.
    H zero-padding is folded into the band matrices, W zero-padding into
    the matmul column ranges.
  - fp32 data is fed to the PE as float32r (full-rate fp32 matmul mode).
  - PSUM -> SBUF eviction (+ bias) alternates between ScalarE and VectorE,
    and DMA in/out transfers are ~1.6 MB each for near-peak HBM bandwidth.
"""

import numpy as np

import bass_rust
import concourse.bass as bass
import concourse.mybir as mybir
import concourse.tile as tile
from concourse.bass_utils import run_bass_kernel_spmd

F32 = mybir.dt.float32
F32R = mybir.dt.float32r

N_CORES = 8
H = W = 224
M = 112        # output rows per chunk
KR = 113       # input rows per chunk (M + 1 halo row at the image edge)
IMGS = 128     # images per core (2 batches x 64 channels)
IB = 32        # images per block (DMA granularity)
NBLK = IMGS // IB
WP = W + 2     # padded image-row width in SBUF (zero column at each edge)
NWIN = 2 * WP - 2  # flat matmul window: 2 images per PSUM group, minus 2
# (r0, i0) per chunk: output-row base and input-row base.
CHUNKS = ((0, 0), (112, 111))

_MAX_WAITS = 1


def _split_multi_waits(nc):
    """Split instructions carrying >1 sync-wait into single-wait NOP
    preludes (the walrus build here rejects multi-wait instructions)."""
    counter = 0
    for fn in nc.m.functions:
        for bb in fn.blocks:
            insts = bb.instructions
            i = 0
            while i < len(insts):
                inst = insts[i]
                si = inst.sync_info
                if si is not None and si.on_wait and len(si.on_wait) > _MAX_WAITS:
                    waits = list(si.on_wait)
                    keep = waits[-_MAX_WAITS:]
                    spill = waits[:-_MAX_WAITS]
                    nops = []
                    for w in spill:
                        nop = mybir.InstNoOp(
                            name=f"waitsplit_{counter}", ins=[], outs=[]
                        )
                        counter += 1
                        nop.engine = inst.engine
                        nop.sync_info = bass_rust.SyncInfo(on_wait=[w], on_update=[])
                        nops.append(nop)
                    inst.sync_info = bass_rust.SyncInfo(
                        on_wait=keep,
                        on_update=list(si.on_update) if si.on_update else [],
                    )
                    insts[i:i] = nops
                    i += len(nops)
                i += 1
    return counter


def build_nc(bias_total: float):
    nc = bass.Bass("TRN2", target_bir_lowering=False, debug=False)
    x_d = nc.dram_tensor("X", [IMGS, H, WP], F32R, kind="ExternalInput").ap()
    bands_d = nc.dram_tensor("BANDS", [2, 3, KR, M], F32R, kind="ExternalInput").ap()
    y_d = nc.dram_tensor("Y", [IMGS, H, W], F32, kind="ExternalOutput").ap()

    with tile.TileContext(nc) as tc:
        with (
            tc.tile_pool(name="const", bufs=1) as cpool,
            tc.tile_pool(name="io", bufs=3) as io_pool,
            tc.tile_pool(name="acc", bufs=8, space="PSUM") as psum_pool,
        ):
            bands = cpool.tile([KR, 2, 3, M], F32R)
            nc.sync.dma_start(bands, bands_d.rearrange("c s k m -> k c s m"))
            ev = 0
            for blk in range(NBLK):
                for c, (r0, i0) in enumerate(CHUNKS):
                    xt = io_pool.tile([KR, IB, WP], F32R, tag="xt")
                    # X arrives host-padded to 226 columns (zero at each
                    # edge), so the DMA delivers the W padding directly.
                    nc.sync.dma_start(
                        xt,
                        x_d[blk * IB:(blk + 1) * IB, i0:i0 + KR, :].rearrange(
                            "i r w -> r i w"
                        ),
                    )
                    xtf = xt.rearrange("k i w -> k (i w)")
                    ot = io_pool.tile([M, IB, W], F32, tag="ot")
                    for p in range(IB // 2):
                        base = 2 * p * WP
                        # One flat 450-wide window per W-shift: fp32r matmuls
                        # need a single even-count free dim and an 8B-aligned
                        # PSUM dst at offset 0, so the dst is always [:, 0:450]
                        # and the W-shift slides the source window. PSUM
                        # columns 224/225 catch the inter-image junk and are
                        # not evicted.
                        ps = psum_pool.tile([M, 2 * WP], F32)
                        for k, dw in enumerate((0, 1, 2)):
                            nc.tensor.matmul(
                                ps[:, 0:NWIN],
                                bands[:, c, dw, :],
                                xtf[:, base + dw:base + dw + NWIN],
                                start=(k == 0),
                                stop=(k == 2),
                            )
                        psv = ps.rearrange("m (i w) -> m i w", w=WP)[:, :, 0:W]
                        dst = ot[:, 2 * p:2 * p + 2, :]
                        if ev % 2 == 0:
                            if bias_total != 0.0:
                                nc.scalar.activation(
                                    dst,
                                    psv,
                                    mybir.ActivationFunctionType.Copy,
                                    bias=float(bias_total),
                                )
                            else:
                                nc.scalar.copy(dst, psv)
                        else:
                            if bias_total != 0.0:
                                nc.vector.tensor_scalar_add(
                                    dst, psv, float(bias_total)
                                )
                            else:
                                nc.vector.tensor_copy(dst, psv)
                        ev += 1
                        # Stores go on the ACT HWDGE ring so the next
                        # block's load (SP ring) never queues behind this
                        # store's eviction wait; two half-stores per block
                        # let the store pipeline start after 4 evictions.
                        if p % 4 == 3:
                            h0 = (p - 3) * 2
                            nc.scalar.dma_start(
                                y_d[
                                    blk * IB + h0:blk * IB + h0 + 8,
                                    r0:r0 + M,
                                    :,
                                ].rearrange("i r w -> r i w"),
                                ot[:, h0:h0 + 8, :],
                            )
    _split_multi_waits(nc)
    return nc


def build_bands(Keff: np.ndarray) -> np.ndarray:
    """Banded H-contraction matrices, [chunk, dw, KR, M] fp32.

    B[c, dw, i, m] = Keff[dh, dw] where input-row index i corresponds to
    absolute row i0 + i and output row r0 + m needs absolute row
    r0 + m + dh - 1; rows outside [0, H) are dropped (zero padding).
    """
    bands = np.zeros((2, 3, KR, M), dtype=np.float32)
    for c, (r0, i0) in enumerate(CHUNKS):
        for dw in range(3):
            for m in range(M):
                for dh in range(3):
                    arow = r0 + m + dh - 1
                    if 0 <= arow < H:
                        bands[c, dw, arow - i0, m] = Keff[dh, dw]
    return bands


_cache = {}


def kernel(X, K, b, padding, stride) -> np.ndarray:
    X = np.ascontiguousarray(np.asarray(X, dtype=np.float32))
    K = np.asarray(K, dtype=np.float32)
    b = np.asarray(b, dtype=np.float32)
    assert int(padding) == 1 and int(stride) == 1, (padding, stride)
    bx, cx, hx, wx = X.shape
    assert (bx, cx, hx, wx) == (16, 64, H, W), X.shape

    bk, ck, hk, wk = K.shape
    Keff = K.sum(axis=(0, 1), dtype=np.float32)
    bias_total = float(b.reshape(())) * (bk * ck * hk * wk)

    key = (round(bias_total, 12) != 0.0)
    if key not in _cache:
        _cache[key] = build_nc(bias_total)
    nc = _cache[key]

    bands = build_bands(Keff)
    Xf = X.reshape(bx * cx, hx, wx)
    Xp = np.zeros((bx * cx, hx, WP), dtype=np.float32)
    Xp[:, :, 1:1 + W] = Xf
    in_maps = [
        {
            "X": Xp[k * IMGS:(k + 1) * IMGS],
            "BANDS": bands,
        }
        for k in range(N_CORES)
    ]
    res = run_bass_kernel_spmd(nc, in_maps, core_ids=list(range(N_CORES)))
    out = np.concatenate([r["Y"] for r in res.results], axis=0)
    return out.reshape(bx, cx, hx, wx)



# revision 2
# speedup vs baseline: 1.5368x; 1.5368x over previous
"""Trainium2 Bass kernel for nn_Conv_39273180955618.

The reference op reduces to a depthwise correlation: every image (batch x
channel plane) of X is correlated with the same 3x3 kernel
Keff = K.sum((0,1)), plus a scalar bias b * prod(K.shape).

Strategy (8 NeuronCores, data-parallel over batch):
  - core k gets batches [2k, 2k+2) = 128 images of 224x224.
  - All device I/O is bf16 (tolerance is 2e-2 scale-relative; measured
    end-to-end bf16 error is ~7e-3), halving HBM traffic vs fp32.
  - The host pre-transposes X into the exact SBUF layout
    [blk, chunk, row, image, width] so every DMA descriptor is a
    14.4 KB contiguous run (the cost model penalizes <512B chunks 2x),
    and un-transposes the bf16 output written in device-native layout.
  - The H-convolution is a TensorE matmul contraction over rows with small
    banded matrices B[chunk, dw] (shape [113, 112]): for each of the 3 W
    shifts dw, Z[:, wout] += B^T @ X[rows, win], accumulated in PSUM.
    H zero-padding is folded into the band matrices, W zero-padding into
    the matmul column ranges.
  - PSUM -> SBUF eviction (+ bias) alternates between ScalarE and VectorE.
"""

import numpy as np
import ml_dtypes

import bass_rust
import concourse.bass as bass
import concourse.mybir as mybir
import concourse.tile as tile
from concourse.bass_utils import run_bass_kernel_spmd

F32 = mybir.dt.float32
BF16 = mybir.dt.bfloat16
NP_BF16 = ml_dtypes.bfloat16

N_CORES = 8
H = W = 224
M = 112        # output rows per chunk
KR = 113       # input rows per chunk (M + 1 halo row at the image edge)
IMGS = 128     # images per core (2 batches x 64 channels)
IB = 32        # images per block (DMA granularity)
NBLK = IMGS // IB
WP = W + 2     # padded image-row width in SBUF (zero column at each edge)
NWIN = 2 * WP - 2  # flat matmul window: 2 images per PSUM group, minus 2
# (r0, i0) per chunk: output-row base and input-row base.
CHUNKS = ((0, 0), (112, 111))

_MAX_WAITS = 1


def _split_multi_waits(nc):
    """Split instructions carrying >1 sync-wait into single-wait NOP
    preludes (the walrus build here rejects multi-wait instructions)."""
    counter = 0
    for fn in nc.m.functions:
        for bb in fn.blocks:
            insts = bb.instructions
            i = 0
            while i < len(insts):
                inst = insts[i]
                si = inst.sync_info
                if si is not None and si.on_wait and len(si.on_wait) > _MAX_WAITS:
                    waits = list(si.on_wait)
                    keep = waits[-_MAX_WAITS:]
                    spill = waits[:-_MAX_WAITS]
                    nops = []
                    for w in spill:
                        nop = mybir.InstNoOp(
                            name=f"waitsplit_{counter}", ins=[], outs=[]
                        )
                        counter += 1
                        nop.engine = inst.engine
                        nop.sync_info = bass_rust.SyncInfo(on_wait=[w], on_update=[])
                        nops.append(nop)
                    inst.sync_info = bass_rust.SyncInfo(
                        on_wait=keep,
                        on_update=list(si.on_update) if si.on_update else [],
                    )
                    insts[i:i] = nops
                    i += len(nops)
                i += 1
    return counter


def build_nc(bias_total: float):
    nc = bass.Bass("TRN2", target_bir_lowering=False, debug=False)
    # Host-transposed input: [blk, chunk, row, image, padded-width].
    x_d = nc.dram_tensor(
        "X", [NBLK, 2, KR, IB, WP], BF16, kind="ExternalInput"
    ).ap()
    bands_d = nc.dram_tensor("BANDS", [2, 3, KR, M], BF16, kind="ExternalInput").ap()
    # Device-native output layout: [chunk, blk, row, image, width].
    y_d = nc.dram_tensor("Y", [2, NBLK, M, IB, W], BF16, kind="ExternalOutput").ap()

    with tile.TileContext(nc) as tc:
        with (
            tc.tile_pool(name="const", bufs=1) as cpool,
            tc.tile_pool(name="io", bufs=3) as io_pool,
            tc.tile_pool(name="acc", bufs=8, space="PSUM") as psum_pool,
        ):
            bands = cpool.tile([KR, 2, 3, M], BF16)
            nc.sync.dma_start(bands, bands_d.rearrange("c s k m -> k c s m"))
            ev = 0
            for blk in range(NBLK):
                for c in range(2):
                    xt = io_pool.tile([KR, IB, WP], BF16, tag="xt")
                    # Both sides are a [KR, IB*WP] contiguous run.
                    nc.sync.dma_start(xt, x_d[blk, c])
                    xtf = xt.rearrange("k i w -> k (i w)")
                    ot = io_pool.tile([M, IB, W], BF16, tag="ot")
                    for p in range(IB // 2):
                        base = 2 * p * WP
                        # One flat 450-wide window per W-shift; PSUM columns
                        # 224/225 catch the inter-image junk and are not
                        # evicted.
                        ps = psum_pool.tile([M, 2 * WP], F32)
                        for k, dw in enumerate((0, 1, 2)):
                            nc.tensor.matmul(
                                ps[:, 0:NWIN],
                                bands[:, c, dw, :],
                                xtf[:, base + dw:base + dw + NWIN],
                                start=(k == 0),
                                stop=(k == 2),
                            )
                        psv = ps.rearrange("m (i w) -> m i w", w=WP)[:, :, 0:W]
                        dst = ot[:, 2 * p:2 * p + 2, :]
                        if ev % 2 == 0:
                            if bias_total != 0.0:
                                nc.scalar.activation(
                                    dst,
                                    psv,
                                    mybir.ActivationFunctionType.Copy,
                                    bias=float(bias_total),
                                )
                            else:
                                nc.scalar.copy(dst, psv)
                        else:
                            if bias_total != 0.0:
                                nc.vector.tensor_scalar_add(
                                    dst, psv, float(bias_total)
                                )
                            else:
                                nc.vector.tensor_copy(dst, psv)
                        ev += 1
                        # Stores go on the ACT HWDGE ring so the next
                        # block's load (SP ring) never queues behind this
                        # store's eviction wait; two half-stores per block
                        # let the store pipeline start after 4 evictions.
                        if p % 4 == 3:
                            h0 = (p - 3) * 2
                            nc.scalar.dma_start(
                                y_d[c, blk, :, h0:h0 + 8, :],
                                ot[:, h0:h0 + 8, :],
                            )
    _split_multi_waits(nc)
    return nc


def build_bands(Keff: np.ndarray) -> np.ndarray:
    """Banded H-contraction matrices, [chunk, dw, KR, M] fp32.

    B[c, dw, i, m] = Keff[dh, dw] where input-row index i corresponds to
    absolute row i0 + i and output row r0 + m needs absolute row
    r0 + m + dh - 1; rows outside [0, H) are dropped (zero padding).
    """
    bands = np.zeros((2, 3, KR, M), dtype=np.float32)
    for c, (r0, i0) in enumerate(CHUNKS):
        for dw in range(3):
            for m in range(M):
                for dh in range(3):
                    arow = r0 + m + dh - 1
                    if 0 <= arow < H:
                        bands[c, dw, arow - i0, m] = Keff[dh, dw]
    return bands


_cache = {}


def kernel(X, K, b, padding, stride) -> np.ndarray:
    X = np.ascontiguousarray(np.asarray(X, dtype=np.float32))
    K = np.asarray(K, dtype=np.float32)
    b = np.asarray(b, dtype=np.float32)
    assert int(padding) == 1 and int(stride) == 1, (padding, stride)
    bx, cx, hx, wx = X.shape
    assert (bx, cx, hx, wx) == (16, 64, H, W), X.shape

    bk, ck, hk, wk = K.shape
    Keff = K.sum(axis=(0, 1), dtype=np.float32)
    bias_total = float(b.reshape(())) * (bk * ck * hk * wk)

    key = (round(bias_total, 12) != 0.0)
    if key not in _cache:
        _cache[key] = build_nc(bias_total)
    nc = _cache[key]

    bands = build_bands(Keff).astype(NP_BF16)

    # Host-side pad + transpose into the device layout
    # [blk, chunk, row, image, padded-width], bf16.
    Xf = X.reshape(bx * cx, hx, wx)
    Xp = np.zeros((bx * cx, hx, WP), dtype=NP_BF16)
    Xp[:, :, 1:1 + W] = Xf.astype(NP_BF16)
    in_maps = []
    for k in range(N_CORES):
        Xc = Xp[k * IMGS:(k + 1) * IMGS].reshape(NBLK, IB, H, WP)
        arr = np.empty((NBLK, 2, KR, IB, WP), dtype=NP_BF16)
        for c, (r0, i0) in enumerate(CHUNKS):
            arr[:, c] = Xc[:, :, i0:i0 + KR, :].transpose(0, 2, 1, 3)
        in_maps.append({"X": arr, "BANDS": bands})

    res = run_bass_kernel_spmd(nc, in_maps, core_ids=list(range(N_CORES)))

    # Un-transpose [chunk, blk, row, image, width] -> [image, h, w], fp32.
    out = np.empty((bx * cx, hx, wx), dtype=np.float32)
    for k in range(N_CORES):
        y = np.asarray(res.results[k]["Y"])  # [2, NBLK, M, IB, W] bf16
        y = y.transpose(1, 3, 0, 2, 4).reshape(IMGS, hx, wx)
        out[k * IMGS:(k + 1) * IMGS] = y.astype(np.float32)
    return out.reshape(bx, cx, hx, wx)


# revision 26
# speedup vs baseline: 1.9689x; 1.2812x over previous
"""Trainium2 Bass kernel for nn_Conv_39273180955618.

The reference op reduces to a depthwise correlation: every image (batch x
channel plane) of X is correlated with the same 3x3 kernel
Keff = K.sum((0,1)), plus a scalar bias b * prod(K.shape).

Strategy (8 NeuronCores, data-parallel over batch):
  - core k gets batches [2k, 2k+2) = 128 images of 224x224, treated as one
    28672-row strip (cross-image H-padding is handled by zeroed band
    coefficients, so chunks may span image boundaries).
  - Strip chunks of 126 output rows use all 128 SBUF partitions (126 + 2
    halo rows), cutting TensorE cycles 12% vs per-image 112-row chunks.
    Band phase repeats every 16 chunks (126*16 = 9*224).
  - Input crosses HBM as int8 (host-quantized, scale folded into the band
    matrices; measured end-to-end error ~9e-3 vs the 2e-2 gate), output as
    bf16 - 19.4 MB/core total vs 51.6 MB for fp32.
  - int8 -> bf16 expansion and PSUM eviction rotate across Scalar/Vector/
    GpSimd so no pointwise engine exceeds ~45us.
  - Host pre-gathers the strip into the exact SBUF tile layout (1.8KB+
    contiguous DMA descriptors) and un-transposes the bf16 output.
"""

import numpy as np
import ml_dtypes

import bass_rust
import concourse.bass as bass
import concourse.mybir as mybir
import concourse.tile as tile
from concourse.bass_utils import run_bass_kernel_spmd

F32 = mybir.dt.float32
BF16 = mybir.dt.bfloat16
I8 = mybir.dt.int8
NP_BF16 = ml_dtypes.bfloat16

N_CORES = 8
H = W = 224
WPAD = W + 2      # padded row width (zero column at each edge)
IMGS = 128        # images per core
NROWS = IMGS * H  # strip rows per core
CO = 126          # output rows per chunk
CI = 128          # input rows per chunk (CO + 2 halo)
NCHUNK = 228      # ceil(NROWS / CO)
NPH = 16          # band phase period: 126*16 == 9*224
CPT = 8           # chunks per DMA tile
TILES = 29        # ceil(NCHUNK / CPT); last tile holds 4 chunks
PAD_SLOTS = TILES * CPT  # 232 gather slots (228 real)

_MAX_WAITS = 1


def _split_multi_waits(nc):
    """Split instructions carrying >1 sync-wait into single-wait NOP
    preludes (the walrus build here rejects multi-wait instructions)."""
    counter = 0
    for fn in nc.m.functions:
        for bb in fn.blocks:
            insts = bb.instructions
            i = 0
            while i < len(insts):
                inst = insts[i]
                si = inst.sync_info
                if si is not None and si.on_wait and len(si.on_wait) > _MAX_WAITS:
                    waits = list(si.on_wait)
                    keep = waits[-_MAX_WAITS:]
                    spill = waits[:-_MAX_WAITS]
                    nops = []
                    for w in spill:
                        nop = mybir.InstNoOp(
                            name=f"waitsplit_{counter}", ins=[], outs=[]
                        )
                        counter += 1
                        nop.engine = inst.engine
                        nop.sync_info = bass_rust.SyncInfo(on_wait=[w], on_update=[])
                        nops.append(nop)
                    inst.sync_info = bass_rust.SyncInfo(
                        on_wait=keep,
                        on_update=list(si.on_update) if si.on_update else [],
                    )
                    insts[i:i] = nops
                    i += len(nops)
                i += 1
    return counter


def build_nc(bias_total: float):
    nc = bass.Bass("TRN2", target_bir_lowering=False, debug=False)
    x_d = nc.dram_tensor(
        "X", [TILES, CI, CPT, WPAD], I8, kind="ExternalInput"
    ).ap()
    # Tile 0 ships pre-converted so the first matmuls skip the convert
    # latency entirely.
    x0_d = nc.dram_tensor(
        "X0", [CI, CPT, WPAD], BF16, kind="ExternalInput"
    ).ap()
    bands_d = nc.dram_tensor(
        "BANDS", [CI, NPH, 3, CO], BF16, kind="ExternalInput"
    ).ap()
    y_d = nc.dram_tensor("Y", [CO, NCHUNK, W], BF16, kind="ExternalOutput").ap()

    # Deterministic engine dedication (greedy balancing causes queue-order
    # priority inversions). GpSimd cannot touch PSUM, so evictions strictly
    # alternate ACT/DVE; converts lean on GpSimd (first half) with the
    # second half alternating ACT/DVE. Each engine lands near 42us, below
    # the 64us TensorE critical path.
    conv_rr = [0]
    evict_rr = [0]
    EVICT_PATTERN = (0, 1)
    CONV_PATTERN = (2, 0, 2, 1)

    def evict(dst, src, engine=None):
        if engine is None:
            eng = EVICT_PATTERN[evict_rr[0] % len(EVICT_PATTERN)]
            evict_rr[0] += 1
        else:
            eng = engine
        if eng == 0:
            if bias_total != 0.0:
                nc.scalar.activation(
                    dst, src, mybir.ActivationFunctionType.Copy,
                    bias=float(bias_total),
                )
            else:
                nc.scalar.copy(dst, src)
        else:
            e = nc.vector if eng == 1 else nc.gpsimd
            if bias_total != 0.0:
                e.tensor_scalar_add(dst, src, float(bias_total))
            else:
                e.tensor_copy(dst, src)

    def convert(dst, src):
        eng = CONV_PATTERN[conv_rr[0] % len(CONV_PATTERN)]
        conv_rr[0] += 1
        if eng == 0:
            nc.scalar.copy(dst, src)
        elif eng == 1:
            nc.vector.tensor_copy(dst, src)
        else:
            nc.gpsimd.tensor_copy(dst, src)

    with tile.TileContext(nc) as tc:
        with (
            tc.tile_pool(name="const", bufs=1) as cpool,
            tc.tile_pool(name="xi", bufs=6) as xi_pool,
            tc.tile_pool(name="xc", bufs=5) as xc_pool,
            tc.tile_pool(name="ot", bufs=4) as ot_pool,
            tc.tile_pool(name="acc", bufs=4, space="PSUM") as psum_pool,
        ):
            bands = cpool.tile([CI, NPH, 3, CO], BF16)
            # Band phases stream just-in-time around the first X tiles
            # (chunk i uses phase i%16, ~280ns of PE work per chunk).
            nc.sync.dma_start(bands[:, 0:2], bands_d[:, 0:2])

            xis, xcs = {}, {}

            def stage_load(g):
                if g == 0:
                    xc = xc_pool.tile([CI, CPT, WPAD], BF16, tag="xc")
                    nc.sync.dma_start(xc, x0_d)
                    xcs[0] = xc
                    nc.sync.dma_start(bands[:, 2:8], bands_d[:, 2:8])
                    return
                xi = xi_pool.tile([CI, CPT, WPAD], I8, tag="xi")
                nc.sync.dma_start(xi, x_d[g])
                xis[g] = xi
                if g == 1:
                    nc.sync.dma_start(bands[:, 8:16], bands_d[:, 8:16])

            def stage_convert(g):
                if g == 0:
                    return
                cpt = CPT if g < TILES - 1 else NCHUNK - CPT * (TILES - 1)
                xi = xis.pop(g)
                xc = xc_pool.tile([CI, CPT, WPAD], BF16, tag="xc")
                half = cpt // 2
                convert(xc[:, 0:half], xi[:, 0:half])
                convert(xc[:, half:cpt], xi[:, half:cpt])
                xcs[g] = xc

            def stage_compute(g):
                cpt = CPT if g < TILES - 1 else NCHUNK - CPT * (TILES - 1)
                xc = xcs.pop(g)
                # [partition, pair-of-chunks, 2, W] so quad evictions get
                # structurally matching access patterns.
                ot = ot_pool.tile([CO, CPT // 2, 2, W], BF16, tag="ot")
                last = g == TILES - 1
                for u in range(cpt // 4):
                    # One PSUM tile spans 2 banks; each chunk's 224-col
                    # accumulator stays inside a single bank.
                    ps = psum_pool.tile([CO, 2, 512], F32)
                    for q in range(2):
                        for hh in range(2):
                            chunk = g * CPT + 4 * u + 2 * q + hh
                            ph = chunk % NPH
                            for k, dw in enumerate((0, 1, 2)):
                                nc.tensor.matmul(
                                    ps[:, q, hh * W:(hh + 1) * W],
                                    bands[:, ph, dw, :],
                                    xc[:, 4 * u + 2 * q + hh, dw:dw + W],
                                    start=(k == 0),
                                    stop=(k == 2),
                                )
                    psv = ps[:, :, 0:2 * W].rearrange(
                        "m a (b w) -> m a b w", w=W
                    )
                    dst = ot[:, 2 * u:2 * u + 2, :, :]
                    if last:
                        # Final quad: fast ACT evictions pair-by-pair, with
                        # stores on the otherwise-idle SP ring, to shorten
                        # the drain tail.
                        for q in range(2):
                            evict(dst[:, q:q + 1], psv[:, q:q + 1], engine=0)
                            nc.sync.dma_start(
                                y_d[
                                    :,
                                    g * CPT + 4 * u + 2 * q:
                                    g * CPT + 4 * u + 2 * q + 2,
                                    :,
                                ].rearrange("m (a b) w -> m a b w", b=2),
                                dst[:, q:q + 1],
                            )
                    else:
                        evict(dst, psv)
                if not last:
                    # Stores ride the SP ring: a store's eviction-wait there
                    # only delays loads (which prefetch 2 tiles ahead), not
                    # the PE-critical converts/evictions on ACT.
                    nc.sync.dma_start(
                        y_d[:, g * CPT:g * CPT + cpt, :].rearrange(
                            "m (a b) w -> m a b w", b=2
                        ),
                        ot[:, 0:cpt // 2, :, :],
                    )

            # Software pipeline: loads run 2 tiles ahead and converts 1
            # tile ahead of compute. Compute (whose evictions gate PSUM
            # reuse and thus the PE) is emitted BEFORE the next convert so
            # each engine queue serves PE-critical evictions first.
            for g in range(TILES + 2):
                if g < TILES:
                    stage_load(g)
                if 0 <= g - 1 < TILES:
                    stage_convert(g - 1)
                if g - 2 >= 0:
                    stage_compute(g - 2)
    _split_multi_waits(nc)
    return nc


def build_bands(Keff_scaled: np.ndarray) -> np.ndarray:
    """Periodic strip band matrices, host layout [CI, NPH, 3, CO] fp32.

    B[k, p, dw, m] = Keff_scaled[dh, dw] with dh = k - m, valid when
    0 <= dh <= 2 and input strip row (126*i - 1 + k) lies in the same
    image as output strip row (126*i + m), evaluated at i = 16 + p
    (the pattern repeats mod 16; image-boundary zeros double as the
    conv's H zero-padding, including the strip edges).
    """
    k = np.arange(CI)[:, None]
    m = np.arange(CO)[None, :]
    dh = k - m
    B = np.zeros((NPH, 3, CI, CO), np.float32)
    for p in range(NPH):
        i = NPH + p
        base_in, base_out = CO * i - 1, CO * i
        mask = (dh >= 0) & (dh <= 2) & (
            (base_in + k) // H == (base_out + m) // H
        )
        for dw in range(3):
            B[p, dw] = np.where(mask, Keff_scaled[np.clip(dh, 0, 2), dw], 0.0)
    return B.transpose(2, 0, 1, 3)  # -> [CI, NPH, 3, CO]


_cache = {}


def kernel(X, K, b, padding, stride) -> np.ndarray:
    X = np.ascontiguousarray(np.asarray(X, dtype=np.float32))
    K = np.asarray(K, dtype=np.float32)
    b = np.asarray(b, dtype=np.float32)
    assert int(padding) == 1 and int(stride) == 1, (padding, stride)
    bx, cx, hx, wx = X.shape
    assert (bx, cx, hx, wx) == (16, 64, H, W), X.shape

    bk, ck, hk, wk = K.shape
    Keff = K.sum(axis=(0, 1), dtype=np.float32)
    bias_total = float(b.reshape(())) * (bk * ck * hk * wk)

    key = (round(bias_total, 12) != 0.0)
    if key not in _cache:
        _cache[key] = build_nc(bias_total)
    nc = _cache[key]

    # Host quantization: X -> int8 with a global scale, folded into bands.
    s = 127.0 / max(float(np.abs(X).max()), 1e-30)
    X8 = np.clip(np.rint(X * s), -127, 127).astype(np.int8)
    bands = build_bands(Keff / s).astype(NP_BF16)

    X8 = X8.reshape(bx * cx, hx, wx)
    gather_rows = CO * (PAD_SLOTS - 1) + CI
    idx = (CO * np.arange(PAD_SLOTS)[:, None] + np.arange(CI)[None, :])
    in_maps = []
    for c in range(N_CORES):
        Xp8 = np.zeros((IMGS, H, WPAD), np.int8)
        Xp8[:, :, 1:1 + W] = X8[c * IMGS:(c + 1) * IMGS]
        Sz = np.zeros((gather_rows + 1, WPAD), np.int8)
        Sz[1:1 + NROWS] = Xp8.reshape(NROWS, WPAD)
        G = Sz[idx]  # [PAD_SLOTS, CI, WPAD]
        arr = np.ascontiguousarray(
            G.reshape(TILES, CPT, CI, WPAD).transpose(0, 2, 1, 3)
        )
        in_maps.append({
            "X": arr,
            "X0": arr[0].astype(NP_BF16),
            "BANDS": bands,
        })

    res = run_bass_kernel_spmd(nc, in_maps, core_ids=list(range(N_CORES)))

    out = np.empty((bx * cx, hx, wx), dtype=np.float32)
    for c in range(N_CORES):
        y = np.asarray(res.results[c]["Y"])  # [CO, NCHUNK, W] bf16
        y = y.transpose(1, 0, 2).reshape(NCHUNK * CO, W)[:NROWS]
        out[c * IMGS:(c + 1) * IMGS] = (
            y.astype(np.float32).reshape(IMGS, hx, wx)
        )
    return out.reshape(bx, cx, hx, wx)
